# revision 1
# baseline (speedup 1.0000x reference)
"""Trainium2 Bass kernel for nn_AbsGlobalHeadProbEncoder (MFVI message passing).

kernel(**inputs) takes the FULL inputs
    x       [4, 1024, 128] f32
    mask    [4, 1024]      i32   (all ones per the problem spec)
    ternary [128, 128, 8]  f32
    global_ [64, 128, 8]   f32
and returns the FULL output [4, 1024, 128] f32.

Sharding: 8 NeuronCores, one batch element per core pair (cores 2n and 2n+1
redundantly compute batch n with all 8 heads). A cross-core AllReduce variant
was measured at ~2.2 ms per 512 KB pairwise all-reduce on this stack - far
more than the ~0.15 ms of per-iteration compute it would save - so full
replication wins.

Per core and MFVI iteration: scores F_H = [H, L, L+G] are built chunk-wise
with fp16 matmuls into fp32 PSUM, softmax'd with a fused exp+rowsum on the
scalar engine, normalized by a per-partition DVE scale, transposed on the PE
(fp16, packed PSUM banks) for the msg_i contraction, and all three messages
accumulate into one shared [D, L] fp32 PSUM region.
"""
import sys
import contextlib

if '/opt/trn_rl_repo' not in sys.path:
    sys.path.insert(0, '/opt/trn_rl_repo')

import numpy as np
import concourse.bacc as bacc
import concourse.mybir as mybir
import concourse.tile as tile
from concourse.masks import make_identity

F32 = mybir.dt.float32
F16 = mybir.dt.float16
AF = mybir.ActivationFunctionType

B = 4
L = 1024
D = 128
H = 8
G = 64
IC = L // 128
N_CORES = 8

_CACHE = {}


def build_kernel(n_iter=4, num_devices=8, groups=None, hpc=4, sb_bufs=2, use_cc=True):
    if groups is None:
        groups = [[2 * i, 2 * i + 1] for i in range(num_devices // 2)]
    nc = bacc.Bacc("TRN2", target_bir_lowering=False, debug=False,
                   num_devices=num_devices)

    xT = nc.declare_dram_parameter("xT", [D, L], F32, isOutput=False)
    tern_a = nc.declare_dram_parameter("tern_a", [D, hpc * D], F16, isOutput=False)
    tern_b = nc.declare_dram_parameter("tern_b", [D, hpc * D], F16, isOutput=False)
    glT = nc.declare_dram_parameter("glT", [D, hpc * G], F16, isOutput=False)
    gl = nc.declare_dram_parameter("gl", [G, hpc * D], F16, isOutput=False)
    y = nc.declare_dram_parameter("y", [L, D], F32, isOutput=True)

    with tile.TileContext(nc) as tc:
        with contextlib.ExitStack() as ctx:
            singles = ctx.enter_context(tc.tile_pool(name="singles", bufs=1))
            sb = ctx.enter_context(tc.tile_pool(name="sb", bufs=sb_bufs))
            hp = ctx.enter_context(tc.tile_pool(name="hp", bufs=2))
            # PSUM budget (8 banks): msgp 2 + fh 2x2 + aux 2x1 = 8
            aux = ctx.enter_context(tc.tile_pool(name="aux", bufs=2, space="PSUM"))
            fh = ctx.enter_context(tc.tile_pool(name="fh", bufs=2, space="PSUM"))
            msgp = ctx.enter_context(tc.tile_pool(name="msgp", bufs=1, space="PSUM"))
            dram = ctx.enter_context(tc.tile_pool(name="dram", bufs=2, space="DRAM"))

            # ---- persistent SBUF state ----
            unaryT = singles.tile([D, L], F32)
            nc.sync.dma_start(unaryT[:], xT[:])
            ta_sb = singles.tile([D, hpc * D], F16)
            nc.sync.dma_start(ta_sb[:], tern_a[:])
            tb_sb = singles.tile([D, hpc * D], F16)
            nc.sync.dma_start(tb_sb[:], tern_b[:])
            glT_sb = singles.tile([D, hpc * G], F16)
            nc.sync.dma_start(glT_sb[:], glT[:])
            gl_sb = singles.tile([G, hpc * D], F16)
            nc.sync.dma_start(gl_sb[:], gl[:])
            id16 = singles.tile([128, 128], F16)
            make_identity(nc, id16[:])
            id32 = singles.tile([128, 128], F32)
            make_identity(nc, id32[:])

            qzT = singles.tile([D, L], F16)
            fzT = singles.tile([D, L], F32)

            def z_tail(fzT_src, last=False):
                """qzT <- softmax_D(fzT_src^T)^T ; if last: y <- fzT_src^T."""
                if last:
                    out_sb = sb.tile([128, L], F32, tag="zout")
                    for ic in range(IC):
                        fz_ps = fh.tile([128, 128], F32, tag="fh_ps")
                        nc.tensor.transpose(fz_ps[:], fzT_src[:, ic * 128:(ic + 1) * 128], id32[:])
                        nc.vector.tensor_copy(out_sb[:, ic * 128:(ic + 1) * 128], fz_ps[:])
                        nc.sync.dma_start(y[ic * 128:(ic + 1) * 128, :],
                                          out_sb[:, ic * 128:(ic + 1) * 128])
                    return
                ez = sb.tile([128, L], F32, tag="ez")
                sums = sb.tile([128, IC], F32, tag="zsums")
                for ic in range(IC):
                    cs = slice(ic * 128, (ic + 1) * 128)
                    fz_ps = fh.tile([128, 128], F32, tag="fh_ps")
                    nc.tensor.transpose(fz_ps[:], fzT_src[:, cs], id32[:])
                    nc.scalar.activation(ez[:, cs], fz_ps[:], AF.Exp,
                                         accum_out=sums[:, ic:ic + 1])
                rz = sb.tile([128, IC], F32, tag="zrz")
                nc.vector.reciprocal(rz[:], sums[:])
                qz_sc = sb.tile([128, L], F16, tag="qzsc")
                for ic in range(IC):
                    cs = slice(ic * 128, (ic + 1) * 128)
                    nc.vector.tensor_scalar_mul(qz_sc[:, cs], ez[:, cs], rz[:, ic:ic + 1])
                qzT_ps = fh.tile([128, L], F16, tag="fh_ps")
                for ic in range(IC):
                    cs = slice(ic * 128, (ic + 1) * 128)
                    nc.tensor.transpose(qzT_ps[:, cs], qz_sc[:, cs], id16[:])
                nc.vector.tensor_copy(qzT[:], qzT_ps[:])

            z_tail(unaryT)

            for it in range(n_iter):
                # ---------- phase A: shared across this core's heads ----------
                s_sb = sb.tile([128, IC * hpc * 128], F16, tag="s_sb")
                r_sb = sb.tile([128, IC * hpc * 128], F16, tag="r_sb")
                for c in range(IC):
                    cs = slice(c * 128, (c + 1) * 128)
                    os_ = slice(c * hpc * 128, (c + 1) * hpc * 128)
                    s_ps = fh.tile([128, hpc * 128], F32, tag="fh_ps")
                    for half in range(max(1, hpc * 128 // 512)):
                        nh = slice(half * 512, min((half + 1) * 512, hpc * 128))
                        nc.tensor.matmul(s_ps[:, nh], qzT[:, cs], ta_sb[:, nh])
                    nc.scalar.copy(s_sb[:, os_], s_ps[:])
                    r_ps = fh.tile([128, hpc * 128], F32, tag="fh_ps")
                    for half in range(max(1, hpc * 128 // 512)):
                        nh = slice(half * 512, min((half + 1) * 512, hpc * 128))
                        nc.tensor.matmul(r_ps[:, nh], qzT[:, cs], tb_sb[:, nh])
                    nc.scalar.copy(r_sb[:, os_], r_ps[:])
                eg_sb = sb.tile([128, IC * hpc * G], F16, tag="eg_sb")
                ics_per_bank = max(1, 512 // (hpc * G))
                for ic2 in range(IC // ics_per_bank):
                    hg_ps = fh.tile([128, ics_per_bank * hpc * G], F32, tag="fh_ps")
                    for k in range(ics_per_bank):
                        ic = ics_per_bank * ic2 + k
                        nc.tensor.matmul(hg_ps[:, k * hpc * G:(k + 1) * hpc * G],
                                         qzT[:, ic * 128:(ic + 1) * 128], glT_sb[:])
                    nc.scalar.activation(
                        eg_sb[:, ic2 * ics_per_bank * hpc * G:(ic2 + 1) * ics_per_bank * hpc * G],
                        hg_ps[:], AF.Exp)
                eg_sums = sb.tile([128, IC * hpc], F32, tag="eg_sums")
                nc.vector.reduce_sum(eg_sums[:],
                                     eg_sb.rearrange("p (s g) -> p s g", g=G),
                                     axis=mybir.AxisListType.X)

                msg_ps = msgp.tile([128, L], F32, tag="msg_ps")

                # ---------- phase B: per head ----------
                for h in range(hpc):
                    hs = slice(h * 128, (h + 1) * 128)
                    st_ps = fh.tile([128, L], F32, tag="fh_ps")
                    for half in range(2):
                        nc.tensor.matmul(st_ps[:, half * 512:(half + 1) * 512],
                                         ta_sb[:, hs], qzT[:, half * 512:(half + 1) * 512])
                    st_sb = hp.tile([128, L], F16, tag="st_sb")
                    nc.scalar.copy(st_sb[:], st_ps[:])

                    e_big = hp.tile([128, IC * L], F16, tag="e_big")
                    et_big = hp.tile([128, IC * L], F16, tag="et_big")
                    sums = hp.tile([128, IC], F32, tag="hsums")
                    tot = hp.tile([128, IC], F32, tag="htot")
                    rr = hp.tile([128, IC], F32, tag="hr")
                    for ic in range(IC):
                        fh_ps = fh.tile([128, L], F32, tag="fh_ps")
                        for half in range(2):
                            nc.tensor.matmul(fh_ps[:, half * 512:(half + 1) * 512],
                                             st_sb[:, ic * 128:(ic + 1) * 128],
                                             qzT[:, half * 512:(half + 1) * 512])
                        nc.scalar.activation(e_big[:, ic * L:(ic + 1) * L], fh_ps[:],
                                             AF.Exp, accum_out=sums[:, ic:ic + 1])
                    eg_h_sums = eg_sums.rearrange("p (s h) -> p s h", h=hpc)[:, :, h]
                    nc.vector.tensor_add(tot[:], sums[:], eg_h_sums)
                    nc.vector.reciprocal(rr[:], tot[:])
                    for ic in range(IC):
                        es = slice(ic * L, (ic + 1) * L)
                        nc.vector.tensor_scalar_mul(e_big[:, es], e_big[:, es],
                                                    rr[:, ic:ic + 1])
                        for half in range(2):
                            nc.tensor.matmul(
                                msg_ps[:, half * 512:(half + 1) * 512],
                                s_sb[:, (ic * hpc + h) * 128:(ic * hpc + h + 1) * 128],
                                e_big[:, ic * L + half * 512: ic * L + (half + 1) * 512],
                                start=(h == 0 and ic == 0), stop=False)
                        t_ps = aux.tile([128, L], F16, tag="aux_ps")
                        for jc in range(IC):
                            nc.tensor.transpose(t_ps[:, jc * 128:(jc + 1) * 128],
                                                e_big[:, ic * L + jc * 128: ic * L + (jc + 1) * 128],
                                                id16[:])
                        cp_eng = nc.vector.tensor_copy
                        cp_eng(
                            et_big.rearrange("p (jc i) -> p jc i", jc=IC)[:, :, ic * 128:(ic + 1) * 128],
                            t_ps.rearrange("p (jc i) -> p jc i", jc=IC))
                    for jc in range(IC):
                        for half in range(2):
                            nc.tensor.matmul(
                                msg_ps[:, half * 512:(half + 1) * 512],
                                r_sb[:, (jc * hpc + h) * 128:(jc * hpc + h + 1) * 128],
                                et_big[:, jc * L + half * 512: jc * L + (half + 1) * 512],
                                start=False, stop=False)
                    egT_ps = aux.tile([64, IC * 128], F16, tag="aux_ps")
                    for ic in range(IC):
                        col = (ic * hpc + h) * G
                        nc.vector.tensor_scalar_mul(eg_sb[:, col:col + G],
                                                    eg_sb[:, col:col + G], rr[:, ic:ic + 1])
                        nc.tensor.transpose(egT_ps[:, ic * 128:(ic + 1) * 128],
                                            eg_sb[:, col:col + G], id16[:])
                    egT_sb = hp.tile([64, IC * 128], F16, tag="egT_sb")
                    nc.scalar.copy(egT_sb[:], egT_ps[:])
                    for half in range(2):
                        nc.tensor.matmul(msg_ps[:, half * 512:(half + 1) * 512],
                                         gl_sb[:, hs],
                                         egT_sb[:, half * 512:(half + 1) * 512],
                                         start=False, stop=(h == hpc - 1))

                # ---------- phase C: all-reduce + Z update ----------
                if use_cc:
                    msg_sb = sb.tile([128, L], F32, tag="msg_sb")
                    nc.vector.tensor_copy(msg_sb[:], msg_ps[:])
                    bi = dram.tile([128, L], F32, tag="cc_in")
                    bo = dram.tile([128, L], F32, tag="cc_out")
                    nc.sync.dma_start(bi[:], msg_sb[:])
                    nc.gpsimd.collective_compute(
                        "AllReduce", mybir.AluOpType.add,
                        replica_groups=groups,
                        ins=[bi.opt()], outs=[bo.opt()])
                    msg_red = sb.tile([128, L], F32, tag="msg_red")
                    nc.sync.dma_start(msg_red[:], bo[:])
                    nc.vector.tensor_add(fzT[:], msg_red[:], unaryT[:])
                else:
                    nc.vector.tensor_add(fzT[:], msg_ps[:], unaryT[:])
                z_tail(fzT, last=(it == n_iter - 1))

    nc.compile()
    return nc

class _Runner:
    """Keeps the jitted SPMD executable alive across kernel() calls."""

    def __init__(self, nc):
        import jax
        from jax.sharding import Mesh, PartitionSpec
        from jax.experimental.shard_map import shard_map
        from concourse.bass2jax import (_bass_exec_p, install_neuronx_cc_hook,
                                        partition_id_tensor)
        install_neuronx_cc_hook()
        self.jax = jax
        in_names, out_names, out_avals, zero_outs = [], [], [], []
        partition_name = nc.partition_id_tensor.name if nc.partition_id_tensor else None
        for alloc in nc.m.functions[0].allocations:
            if not isinstance(alloc, mybir.MemoryLocationSet):
                continue
            name = alloc.memorylocations[0].name
            if alloc.kind == "ExternalInput":
                if name != partition_name:
                    in_names.append(name)
            elif alloc.kind == "ExternalOutput":
                out_names.append(name)
                shape = tuple(alloc.tensor_shape)
                dtype = mybir.dt.np(alloc.dtype)
                out_avals.append(jax.core.ShapedArray(shape, dtype))
                zero_outs.append(np.zeros(shape, dtype))
        self.in_names, self.out_names = in_names, out_names
        self.out_avals, self.zero_outs = out_avals, zero_outs
        all_in_names = list(in_names) + list(out_names)
        if partition_name is not None:
            all_in_names.append(partition_name)

        def _body(*args):
            operands = list(args)
            if partition_name is not None:
                operands.append(partition_id_tensor())
            outs = _bass_exec_p.bind(
                *operands,
                out_avals=tuple(out_avals),
                in_names=tuple(all_in_names),
                out_names=tuple(out_names),
                lowering_input_output_aliases=(),
                sim_require_finite=True,
                sim_require_nnan=True,
                nc=nc,
            )
            return tuple(outs)

        devices = jax.devices()[:N_CORES]
        mesh = Mesh(np.asarray(devices), ("core",))
        n_params = len(in_names)
        in_specs = (PartitionSpec("core"),) * (n_params + len(out_names))
        out_specs = (PartitionSpec("core"),) * len(out_names)
        self.fn = jax.jit(shard_map(_body, mesh=mesh, in_specs=in_specs,
                                    out_specs=out_specs, check_rep=False),
                          keep_unused=True)

    def __call__(self, in_maps):
        jax = self.jax
        concat_in = [
            np.concatenate([np.asarray(in_maps[c][name]) for c in range(N_CORES)], axis=0)
            for name in self.in_names
        ]
        concat_zeros = [np.zeros((N_CORES * z.shape[0], *z.shape[1:]), z.dtype)
                        for z in self.zero_outs]
        outs = self.fn(*concat_in, *concat_zeros)
        jax.block_until_ready(outs)
        return [
            {name: np.asarray(outs[i]).reshape(N_CORES, *self.out_avals[i].shape)[c]
             for i, name in enumerate(self.out_names)}
            for c in range(N_CORES)
        ]


def make_core_inputs(x, ternary, global_, core, hpc=8):
    n = core // 2
    if hpc == 8:
        heads = list(range(8))
    else:
        hg = core % 2
        heads = list(range(hg * hpc, (hg + 1) * hpc))
    t = ternary[:, :, heads]
    g = global_[:, :, heads]
    return {
        "xT": np.ascontiguousarray(x[n].T.astype(np.float32)),
        "tern_a": np.ascontiguousarray(t.transpose(0, 2, 1).reshape(D, hpc * D).astype(np.float16)),
        "tern_b": np.ascontiguousarray(t.transpose(1, 2, 0).reshape(D, hpc * D).astype(np.float16)),
        "glT": np.ascontiguousarray(g.transpose(1, 2, 0).reshape(D, hpc * G).astype(np.float16)),
        "gl": np.ascontiguousarray(g.transpose(0, 2, 1).reshape(G, hpc * D).astype(np.float16)),
    }


def get_runner(n_iter=4):
    key = ("runner", n_iter)
    if key not in _CACHE:
        nc = build_kernel(n_iter=n_iter, num_devices=N_CORES, hpc=8, use_cc=False)
        _CACHE[key] = _Runner(nc)
    return _CACHE[key]


def kernel(x, mask, ternary, global_):
    x = np.asarray(x, dtype=np.float32)
    mask = np.asarray(mask)
    ternary = np.asarray(ternary, dtype=np.float32)
    global_ = np.asarray(global_, dtype=np.float32)

    run = get_runner(4)
    in_maps = [make_core_inputs(x, ternary, global_, c) for c in range(N_CORES)]
    res = run(in_maps)
    out = np.stack([res[2 * n]["y"] for n in range(B)])
    out = np.where((mask != 0)[..., None], out, np.float32(0.0)).astype(np.float32)
    return out

